# revision 22
# baseline (speedup 1.0000x reference)
"""Trainium2 Bass kernel for nn_AttentionModule_39616778338491 (chord sparse attention).

Structure: V = gMLP(V); 12x { W = fMLP_m(input); V = chord_spmm(W, V) + V }.

Sharding (8 cores): pair (2b, 2b+1) owns batch b. Within a pair, MLPs are
row-split (each core computes its 2048 rows); the chord chain is E-split
(each core computes all 4096 rows x its 128-column half) -- the E-split
keeps the chord gather local (no per-layer halo) and halves chord PE work.
An AllToAll on the pair transposes g(V) from row-split to column-split
without the program ever referencing its core id (SPMD-symmetric); per-layer
AllGathers share the small W link-weight tensors.

Fused pipeline: step m runs f-MLP_m (PE+ACT heavy) interleaved with chord
layer m-2 (PE+DVE) while layer m-1's S diagonal images rebuild on the DMA
queues (DRAM staging for the skewed diagonal writes, then bulk reload into
parity-double-buffered SBUF tiles). This hides the S reload (~6MB/layer),
the 13 collectives, and keeps the PE continuously busy (full p-state).

Chord spmm as dense PE matmuls: for each 128-row output block the 13
power-of-2 offsets touch 6 source blocks {+0,+1,+2,+4,+8,+16}; per-row
weights are embedded as diagonals of six 128x128 bf16 lhsT tiles. Output
blocks are processed in pairs sharing one [128, 256] PSUM tile so the f32
residual add (DVE) and the bf16 recast for the next layer batch two blocks
per instruction.
"""

import os
import numpy as np

B, N, E, H = 4, 4096, 256, 1024
E2 = E // 2           # column half per core for the chord
NW = 12
NL = 13
OFFS = [0, 1, 2, 4, 8, 16, 32, 64, 128, 256, 512, 1024, 2048]
ROWS = N // 2          # rows per core for MLP work
NBLK = N // 128        # 32 blocks of 128 rows
CH = 512               # row-chunk for MLP matmuls
NCH = ROWS // CH
HT = H // 128          # 8 h-tiles
PITCH = NBLK * 128     # free width of an S tile (elems)
GROUPS = [[0, 1], [2, 3], [4, 5], [6, 7]]


def _install_patches():
    """Walrus in this image rejects >1 sem wait on the Tile tail Drain;
    spread the waits across preceding sync-engine nops. Also raise the
    stale SBUF cap (207.87 KB/partition is the real limit here)."""
    import concourse.mybir as mybir
    from concourse.tile import TileContext
    from concourse.vector_clock import ScopedClock
    from concourse import tile_utils

    def _dab(self, tick_clock, wait_clock):
        nops = [self.nc.sync.nop(nofuse=True) for _ in range(27)]
        drain_inst = self.nc.sync.drain()
        wait_clock.add_sem_waits(
            drain_inst.ins, ScopedClock({None: tick_clock.global_clock})
        )
        si = drain_inst.ins.sync_info
        waits = list(si.on_wait) if si else []
        if len(waits) > 1:
            si.on_wait.clear()
            si.on_wait.append(waits[0])
            for w, nop in zip(waits[1:], nops):
                nsi = nop.ins.sync_info
                if nsi is None:
                    nop.ins.sync_info = mybir.SyncInfo(on_update=[], on_wait=[w])
                else:
                    nsi.on_wait.append(w)
        self.nc.all_engine_barrier()
        popped = self.nc._tile_sem_poison_stack.pop()
        assert popped is self._sem_poison
        self.nc.clear_and_free_semaphores(list(self.sems.allocated().values()))
        self.nc.all_engine_barrier()

    TileContext._drain_and_barrier = _dab
    tile_utils.max_sbuf_usage = 206 * 1024


def _split_multi_waits(nc, mybir, limit=1):
    """This walrus build accepts at most one sem wait per instruction;
    hoist extra waits onto same-engine NoOps inserted just before."""
    uid = 0
    for f in nc.m.functions:
        for bb in f.blocks:
            new = []
            for inst in bb.instructions:
                si = inst.sync_info
                waits = list(si.on_wait) if si and si.on_wait else []
                if len(waits) > limit:
                    for w in waits[:-limit]:
                        nop = mybir.InstNoOp(name=f"waitsplit-{uid}", ins=[], outs=[])
                        uid += 1
                        nop.engine = inst.engine
                        nop.sync_info = mybir.SyncInfo(on_update=[], on_wait=[w])
                        new.append(nop)
                    si.on_wait.clear()
                    si.on_wait.append(waits[-1])
                new.append(inst)
            bb.instructions = new


def _build_program(nw):
    import bass_rust
    import concourse.bass as bass
    import concourse.mybir as mybir
    from concourse.tile import TileContext

    f32 = mybir.dt.float32
    bf16 = mybir.dt.bfloat16
    AF = mybir.ActivationFunctionType
    V64 = bass_rust.VecI64Pair

    nc = bass.Bass()
    vt = nc.declare_dram_parameter("vt", [E, ROWS], bf16, isOutput=False)
    inpt = nc.declare_dram_parameter("inpt", [E, ROWS], bf16, isOutput=False)
    gw1 = nc.declare_dram_parameter("gw1", [E, H], bf16, isOutput=False)
    gw2 = nc.declare_dram_parameter("gw2", [H, E], bf16, isOutput=False)
    gb1t = nc.declare_dram_parameter("gb1t", [128, HT], f32, isOutput=False)
    gb2r = nc.declare_dram_parameter("gb2r", [1, E], bf16, isOutput=False)
    fw1 = nc.declare_dram_parameter("fw1", [nw, E, H], bf16, isOutput=False)
    fw2t = nc.declare_dram_parameter("fw2t", [nw, 128, HT * NL], bf16, isOutput=False)
    fb1t = nc.declare_dram_parameter("fb1t", [128, nw * HT], f32, isOutput=False)
    fb2c = nc.declare_dram_parameter("fb2c", [NL, nw], f32, isOutput=False)
    onesr = nc.declare_dram_parameter("onesr", [1, E], bf16, isOutput=False)
    out = nc.declare_dram_parameter("out", [N, E2], f32, isOutput=True)

    with TileContext(nc) as tc:
        with (
            tc.tile_pool(name="pc", bufs=1) as pc,
            tc.tile_pool(name="pin", bufs=1) as pin,
            tc.tile_pool(name="pfh", bufs=2) as pfh,
            tc.tile_pool(name="pfw1", bufs=2) as pfw1,
            tc.tile_pool(name="pfw2", bufs=2) as pfw2,
            tc.tile_pool(name="pvtc", bufs=2) as pvtc,
            tc.tile_pool(name="ptmp", bufs=1) as ptmp,
            tc.tile_pool(name="pv", bufs=1) as pv,
            tc.tile_pool(name="ps", bufs=1) as ps,
            tc.tile_pool(name="pwp", bufs=1) as pwp,
            tc.tile_pool(name="pdram", bufs=1, space="DRAM") as pdram,
            tc.tile_pool(name="psA", bufs=2, space="PSUM") as psA,
            tc.tile_pool(name="psW", bufs=2, space="PSUM") as psW,
            tc.tile_pool(name="psO", bufs=1, space="PSUM") as psO,
            tc.tile_pool(name="psC", bufs=3, space="PSUM") as psC,
        ):
            # ---------- DRAM staging tiles (Tile-tracked for cc deps) ----
            va_in = pdram.tile([ROWS, E], bf16, tag="va_in", name="va_in")
            va_out = pdram.tile([2, ROWS, E], bf16, tag="va_out", name="va_out")
            wsis = [
                pdram.tile([NL, ROWS], bf16, tag=f"wsi{m}", name=f"wsi{m}")
                for m in range(nw)
            ]
            wsos = [
                pdram.tile([2, NL, ROWS], bf16, tag=f"wso{m}", name=f"wso{m}")
                for m in range(nw)
            ]
            stage = [
                pdram.tile([6 * 128, PITCH], bf16, tag=f"sst{p}", name=f"sst{p}")
                for p in range(2)
            ]

            # ---------- persistent SBUF tiles ----------
            gw1_t = [pc.tile([128, H], bf16, tag=f"gw1_{k}", name=f"gw1_{k}") for k in range(2)]
            gw2_t = pc.tile([128, HT * E], bf16, tag="gw2", name="gw2")
            gb1_t = pc.tile([128, HT], f32, tag="gb1", name="gb1")
            gb2_t = pc.tile([1, E], bf16, tag="gb2", name="gb2")
            ones_t = pc.tile([1, E], bf16, tag="ones", name="ones")
            fb1_t = pc.tile([128, nw * HT], f32, tag="fb1", name="fb1")
            fb2_t = pc.tile([NL, nw], f32, tag="fb2", name="fb2")
            inp_t = [pin.tile([128, ROWS], bf16, tag=f"inp{k}", name=f"inp{k}") for k in range(2)]

            vcur = pv.tile([128, NBLK * E2], f32, tag="va", name="va")
            vnxt = pv.tile([128, NBLK * E2], f32, tag="vb", name="vb")
            vbf = [pv.tile([128, NBLK * E2], bf16, tag=f"vbf{p}", name=f"vbf{p}") for p in range(2)]
            S = [
                [ps.tile([128, PITCH], bf16, tag=f"s{p}_{k}", name=f"s{p}_{k}") for k in range(6)]
                for p in range(2)
            ]
            Wt = pwp.tile([NL, N], bf16, tag="wt", name="wt")
            wt1 = pwp.tile([NL, N], bf16, tag="wt1", name="wt1")

            # ---------- prologue loads (g-critical first) ----------
            for k in range(2):
                nc.sync.dma_start(out=gw1_t[k][:], in_=gw1[k * 128:(k + 1) * 128, :])
            nc.sync.dma_start(out=gb1_t[:], in_=gb1t[:])
            nc.sync.dma_start(out=gb2_t[:], in_=gb2r[:])
            nc.sync.dma_start(out=ones_t[:], in_=onesr[:])
            for t in range(HT):
                nc.sync.dma_start(
                    out=gw2_t[:, t * E:(t + 1) * E], in_=gw2[t * 128:(t + 1) * 128, :]
                )
            for k in range(2):
                nc.sync.dma_start(out=inp_t[k][:], in_=inpt[k * 128:(k + 1) * 128, :])
            nc.sync.dma_start(out=fb1_t[:], in_=fb1t[:])
            nc.sync.dma_start(out=fb2_t[:], in_=fb2c[:])
            nc.vector.memset(vbf[0][:], 0.0)

            def front_ht(w1k, bias_col, rhs_tiles, fh, ht):
                """One H-tile of an MLP front: 2 matmuls + gelu."""
                pa = psA.tile([128, CH], f32, tag="pa", name="pa")
                for kt in range(2):
                    nc.tensor.matmul(
                        pa[:],
                        lhsT=w1k[kt][:, ht * 128:(ht + 1) * 128],
                        rhs=rhs_tiles[kt],
                        start=(kt == 0),
                        stop=(kt == 1),
                    )
                nc.scalar.activation(fh[ht][:], pa[:], AF.Gelu, bias=bias_col(ht))

            def new_fh():
                return [pfh.tile([128, CH], bf16, tag=f"fh{t}", name=f"fh{t}") for t in range(HT)]

            # ---------- g MLP -> va_in (own rows) ----------
            gfh = {}
            for ch in range(NCH):
                vt_c = [pvtc.tile([128, CH], bf16, tag=f"vtc{k}", name=f"vtc{k}") for k in range(2)]
                for k in range(2):
                    nc.sync.dma_start(
                        out=vt_c[k][:], in_=vt[k * 128:(k + 1) * 128, ch * CH:(ch + 1) * CH]
                    )

                def g_back(cb, t):
                    po = psO.tile([128, E], f32, tag="po", name="po")
                    nc.tensor.matmul(
                        po[:], lhsT=ones_t[0:1, 0:128], rhs=gb2_t[0:1, :],
                        start=True, stop=False,
                    )
                    fhb = gfh[cb]
                    for ht2 in range(HT):
                        nc.tensor.matmul(
                            po[:],
                            lhsT=fhb[ht2][:, t * 128:(t + 1) * 128],
                            rhs=gw2_t[:, ht2 * E:(ht2 + 1) * E],
                            start=False,
                            stop=(ht2 == HT - 1),
                        )
                    tmp = ptmp.tile([128, E], bf16, tag="tv", name="tv")
                    nc.scalar.copy(tmp[:], po[:])
                    blk = cb * 4 + t
                    nc.sync.dma_start(
                        out=va_in[blk * 128:(blk + 1) * 128, :], in_=tmp[:]
                    )

                fh = new_fh()
                gfh[ch] = fh
                for ht in range(HT):
                    front_ht(gw1_t, lambda h2: gb1_t[:, h2:h2 + 1], vt_c, fh, ht)
                    if ch >= 1 and ht % 2 == 1:
                        g_back(ch - 1, ht // 2)
            for t in range(4):
                g_back(NCH - 1, t)

            # pair AllGather of g(V); each core keeps its column half below
            nc.gpsimd.collective_compute(
                "AllGather", mybir.AluOpType.bypass, replica_groups=GROUPS,
                ins=[va_in[:]], outs=[va_out[:]],
            )

            for par in range(2):
                for k in range(6):
                    for hf in range(2):
                        cols = slice(hf * (PITCH // 2), (hf + 1) * (PITCH // 2))
                        nc.sync.dma_start(
                            out=stage[par][k * 128:(k + 1) * 128, cols],
                            in_=vbf[0][:, cols],
                        )

            # ---------- per-layer pieces ----------
            def load_fw(m):
                w1 = [pfw1.tile([128, H], bf16, tag=f"fw1_{k}", name=f"fw1_{k}") for k in range(2)]
                for k in range(2):
                    nc.sync.dma_start(out=w1[k][:], in_=fw1[m, k * 128:(k + 1) * 128, :])
                w2 = pfw2.tile([128, HT * NL], bf16, tag="fw2", name="fw2")
                nc.sync.dma_start(out=w2[:], in_=fw2t[m])
                return w1, w2

            def f_back(m, w2, fh, ch):
                """Back projection of one f chunk: 8 matmuls -> +bias -> DRAM."""
                pw = psW.tile([NL, CH], f32, tag="pw", name="pw")
                for ht in range(HT):
                    nc.tensor.matmul(
                        pw[:],
                        lhsT=w2[:, ht * NL:(ht + 1) * NL],
                        rhs=fh[ht][:],
                        start=(ht == 0),
                        stop=(ht == HT - 1),
                    )
                wc = ptmp.tile([NL, CH], bf16, tag="tw", name="tw")
                nc.vector.tensor_scalar_add(wc[:], pw[:], fb2_t[:, m:m + 1])
                nc.sync.dma_start(out=wsis[m][:, ch * CH:(ch + 1) * CH], in_=wc[:])

            def s_prep_a(m):
                """W load + (j, b)-interleave for layer m (Pool engine)."""
                for h2 in range(2):
                    nc.sync.dma_start(
                        out=wt1[:, h2 * ROWS:(h2 + 1) * ROWS], in_=wsos[m][h2]
                    )
                nc.vector.tensor_copy(
                    Wt[:].rearrange("l (j b) -> l j b", b=NBLK),
                    wt1[:].rearrange("l (b j) -> l j b", j=128),
                )

            def s_prep_b(m):
                """Diagonal rewrites of the DRAM staging image + SBUF reload."""
                st = stage[m % 2][:].rearrange("a b -> (a b)")
                Sp = S[m % 2]
                for li, d in enumerate(OFFS):
                    if d <= 128:
                        segs = []
                        if 128 - d > 0:
                            segs.append((0, 0, 128 - d, d))
                        if d > 0:
                            segs.append((1, 128 - d, d, 0))
                    else:
                        si = {256: 2, 512: 3, 1024: 4, 2048: 5}[d]
                        segs = [(si, 0, 128, 0)]
                    for (si, j0, cnt, p0) in segs:
                        src = Wt[li:li + 1, j0 * NBLK:(j0 + cnt) * NBLK]
                        doff = si * 128 * PITCH + p0 * PITCH + j0 * NBLK
                        dst = st[doff:doff + 1]
                        dst.ap = V64([[PITCH + NBLK, cnt], [1, NBLK]])
                        nc.sync.dma_start(out=dst, in_=src)
                for k in range(6):
                    for q4 in range(4):
                        cols = slice(q4 * (PITCH // 4), (q4 + 1) * (PITCH // 4))
                        nc.sync.dma_start(
                            out=Sp[k][:, cols],
                            in_=stage[m % 2][k * 128:(k + 1) * 128, cols],
                        )

            def chord_quad(k, bq, vc, vn):
                """Output blocks (4bq..4bq+3) of chord layer k: 24 matmuls
                into one bank-wide PSUM tile, f32 residual add (DVE), bf16
                recast (Pool). The last layer streams straight to DRAM."""
                Sp = S[k % 2]
                vb = vbf[k % 2]
                po = psC.tile([128, 4 * E2], f32, tag="pq", name="pq")
                for half in range(4):
                    blk = 4 * bq + half
                    srcs = [(0, blk), (1, (blk + 1) % NBLK)]
                    for i, dl in enumerate([2, 4, 8, 16]):
                        srcs.append((2 + i, (blk + dl) % NBLK))
                    for ii, (si, sb) in enumerate(srcs):
                        nc.tensor.matmul(
                            po[:, half * E2:(half + 1) * E2],
                            lhsT=Sp[si][:, blk::NBLK],
                            rhs=vb[:, sb * E2:(sb + 1) * E2],
                            start=(ii == 0),
                            stop=(ii == 5),
                        )
                sl = slice(bq * 4 * E2, (bq + 1) * 4 * E2)
                nc.vector.tensor_add(vn[:, sl], po[:], vc[:, sl])
                if k + 1 < nw:
                    nc.vector.tensor_copy(vbf[(k + 1) % 2][:, sl], vn[:, sl])
                else:
                    nc.sync.dma_start(
                        out=out[4 * bq * 128:(4 * bq + 4) * 128, :].rearrange(
                            "(b p) e -> p b e", p=128
                        ),
                        in_=vn[:, sl].rearrange("p (b e) -> p b e", e=E2),
                    )

            def cc_w(m):
                nc.gpsimd.collective_compute(
                    "AllGather", mybir.AluOpType.bypass, replica_groups=GROUPS,
                    ins=[wsis[m][:]], outs=[wsos[m][:]],
                )

            def f_step(m, w1, w2, quads):
                """One pipeline step: f-MLP_m at chunk granularity with chord
                quads as PE spacers; the W AllGather fires before the
                trailing quads so its latency hides upstream."""
                qi = 0
                for ch in range(NCH):
                    fh = new_fh()
                    rhs = [inp_t[k][:, ch * CH:(ch + 1) * CH] for k in range(2)]
                    for ht in range(HT):
                        front_ht(
                            w1, lambda h2: fb1_t[:, m * HT + h2:m * HT + h2 + 1],
                            rhs, fh, ht,
                        )
                    if ch < NCH - 1 and qi < len(quads):
                        quads[qi]()
                        qi += 1
                    f_back(m, w2, fh, ch)
                    if ch < NCH - 1 and qi < len(quads):
                        quads[qi]()
                        qi += 1
                cc_w(m)
                for q in quads[qi:]:
                    q()

            # ---------- fused pipeline ----------
            # step m: f_m || chord_{m-3} || s_prep(m-2); lag 3 keeps every
            # wait (cc completion, S reload) at least one full step upstream.
            vc, vn = vcur, vnxt
            w1n, w2n = load_fw(0)
            for m in range(nw):
                w1, w2 = w1n, w2n
                if m + 1 < nw:
                    w1n, w2n = load_fw(m + 1)
                if m >= 2:
                    s_prep_a(m - 2)
                quads = []
                if m >= 3:
                    quads = [
                        (lambda k=m - 3, b=bq, c=vc, n=vn: chord_quad(k, b, c, n))
                        for bq in range(8)
                    ]
                f_step(m, w1, w2, quads)
                if m >= 2:
                    s_prep_b(m - 2)
                if m >= 3:
                    vc, vn = vn, vc
                if m == 0:
                    # initial V load: pick my column half of the gathered
                    # g(V) with conditional DMAs keyed on core parity
                    eh = nc.sync.partition_id() % 2
                    dv = vbf[0][:].rearrange("p (blk e) -> p blk e", e=E2)
                    for par in range(2):
                        sv = va_out[:].rearrange("a b c -> (a b c)")[par * E2:par * E2 + 1]
                        sv.ap = V64([[E, 128], [128 * E, NBLK], [1, E2]])
                        nc.sync.dma_start(out=dv, in_=sv, cond=(eh == par))
                    for q in range(4):
                        nc.vector.tensor_copy(
                            vcur[:, q * 8 * E2:(q + 1) * 8 * E2],
                            vbf[0][:, q * 8 * E2:(q + 1) * 8 * E2],
                        )

            # tail: three chord-only steps with next-layer S prep interleaved
            for k in range(nw - 3, nw):
                if k + 1 < nw:
                    s_prep_a(k + 1)
                for bq in range(8):
                    chord_quad(k, bq, vc, vn)
                    if bq == 1 and k + 1 < nw:
                        s_prep_b(k + 1)
                vc, vn = vn, vc

    _split_multi_waits(nc, mybir)
    return nc


def kernel(**inputs):
    _install_patches()
    from concourse.bass_utils import run_bass_kernel_spmd

    nw = int(os.environ.get("K_NW", NW))
    V = np.ascontiguousarray(np.asarray(inputs["V"], dtype=np.float32))
    inp = np.ascontiguousarray(np.asarray(inputs["input"], dtype=np.float32))
    g_W1 = np.ascontiguousarray(np.asarray(inputs["g_W1"], dtype=np.float32))
    g_b1 = np.asarray(inputs["g_b1"], dtype=np.float32)
    g_W2 = np.ascontiguousarray(np.asarray(inputs["g_W2"], dtype=np.float32))
    g_b2 = np.asarray(inputs["g_b2"], dtype=np.float32)
    f_W1 = np.ascontiguousarray(np.asarray(inputs["f_W1"], dtype=np.float32))[:nw]
    f_b1 = np.asarray(inputs["f_b1"], dtype=np.float32)[:nw]
    f_W2 = np.ascontiguousarray(np.asarray(inputs["f_W2"], dtype=np.float32))[:nw]
    f_b2 = np.asarray(inputs["f_b2"], dtype=np.float32)[:nw]

    import ml_dtypes

    bf = ml_dtypes.bfloat16
    gb1t = np.ascontiguousarray(g_b1.reshape(HT, 128).T)
    fw2t = np.ascontiguousarray(
        f_W2.reshape(nw, HT, 128, NL).transpose(0, 2, 1, 3).reshape(nw, 128, HT * NL)
    ).astype(bf)
    fb1t = np.ascontiguousarray(
        f_b1.reshape(nw, HT, 128).transpose(2, 0, 1).reshape(128, nw * HT)
    )
    fb2c = np.ascontiguousarray(f_b2.T)

    shared = {
        "gw1": g_W1.astype(bf),
        "gw2": g_W2.astype(bf),
        "gb1t": gb1t,
        "gb2r": np.ascontiguousarray(g_b2[None, :]).astype(bf),
        "onesr": np.ones((1, E), bf),
        "fw1": f_W1.astype(bf),
        "fw2t": fw2t,
        "fb1t": fb1t,
        "fb2c": fb2c,
    }
    in_maps = []
    for c in range(8):
        b, h = c // 2, c % 2
        rows = slice(h * ROWS, (h + 1) * ROWS)
        m = dict(shared)
        m["vt"] = np.ascontiguousarray(V[b, rows].T).astype(bf)
        m["inpt"] = np.ascontiguousarray(inp[b, rows].T).astype(bf)
        in_maps.append(m)

    nc = _build_program(nw)
    trace = bool(int(os.environ.get("K_TRACE", "0")))
    res = run_bass_kernel_spmd(nc, in_maps, list(range(8)), trace=trace)
    kernel.last_result = res

    outp = np.empty((B, N, E), np.float32)
    for b in range(B):
        outp[b, :, 0:E2] = res.results[2 * b]["out"]
        outp[b, :, E2:E] = res.results[2 * b + 1]["out"]
    return outp


# revision 23
# speedup vs baseline: 1.0344x; 1.0344x over previous
"""Trainium2 Bass kernel for nn_AttentionModule_39616778338491 (chord sparse attention).

Structure: V = gMLP(V); 12x { W = fMLP_m(input); V = chord_spmm(W, V) + V }.

Sharding (8 cores): pair (2b, 2b+1) owns batch b. Within a pair, MLPs are
row-split (each core computes its 2048 rows); the chord chain is E-split
(each core computes all 4096 rows x its 128-column half) -- the E-split
keeps the chord gather local (no per-layer halo) and halves chord PE work.
An AllToAll on the pair transposes g(V) from row-split to column-split
without the program ever referencing its core id (SPMD-symmetric); per-layer
AllGathers share the small W link-weight tensors.

Fused pipeline: step m runs f-MLP_m (PE+ACT heavy) interleaved with chord
layer m-2 (PE+DVE) while layer m-1's S diagonal images rebuild on the DMA
queues (DRAM staging for the skewed diagonal writes, then bulk reload into
parity-double-buffered SBUF tiles). This hides the S reload (~6MB/layer),
the 13 collectives, and keeps the PE continuously busy (full p-state).

Chord spmm as dense PE matmuls: for each 128-row output block the 13
power-of-2 offsets touch 6 source blocks {+0,+1,+2,+4,+8,+16}; per-row
weights are embedded as diagonals of six 128x128 bf16 lhsT tiles. Output
blocks are processed in pairs sharing one [128, 256] PSUM tile so the f32
residual add (DVE) and the bf16 recast for the next layer batch two blocks
per instruction.
"""

import os
import numpy as np

B, N, E, H = 4, 4096, 256, 1024
E2 = E // 2           # column half per core for the chord
NW = 12
NL = 13
OFFS = [0, 1, 2, 4, 8, 16, 32, 64, 128, 256, 512, 1024, 2048]
ROWS = N // 2          # rows per core for MLP work
NBLK = N // 128        # 32 blocks of 128 rows
CH = 512               # row-chunk for MLP matmuls
NCH = ROWS // CH
HT = H // 128          # 8 h-tiles
PITCH = NBLK * 128     # free width of an S tile (elems)
GROUPS = [[0, 1], [2, 3], [4, 5], [6, 7]]


def _install_patches():
    """Walrus in this image rejects >1 sem wait on the Tile tail Drain;
    spread the waits across preceding sync-engine nops. Also raise the
    stale SBUF cap (207.87 KB/partition is the real limit here)."""
    import concourse.mybir as mybir
    from concourse.tile import TileContext
    from concourse.vector_clock import ScopedClock
    from concourse import tile_utils

    def _dab(self, tick_clock, wait_clock):
        nops = [self.nc.sync.nop(nofuse=True) for _ in range(27)]
        drain_inst = self.nc.sync.drain()
        wait_clock.add_sem_waits(
            drain_inst.ins, ScopedClock({None: tick_clock.global_clock})
        )
        si = drain_inst.ins.sync_info
        waits = list(si.on_wait) if si else []
        if len(waits) > 1:
            si.on_wait.clear()
            si.on_wait.append(waits[0])
            for w, nop in zip(waits[1:], nops):
                nsi = nop.ins.sync_info
                if nsi is None:
                    nop.ins.sync_info = mybir.SyncInfo(on_update=[], on_wait=[w])
                else:
                    nsi.on_wait.append(w)
        self.nc.all_engine_barrier()
        popped = self.nc._tile_sem_poison_stack.pop()
        assert popped is self._sem_poison
        self.nc.clear_and_free_semaphores(list(self.sems.allocated().values()))
        self.nc.all_engine_barrier()

    TileContext._drain_and_barrier = _dab
    tile_utils.max_sbuf_usage = 206 * 1024


def _split_multi_waits(nc, mybir, limit=1):
    """This walrus build accepts at most one sem wait per instruction;
    hoist extra waits onto same-engine NoOps inserted just before."""
    uid = 0
    for f in nc.m.functions:
        for bb in f.blocks:
            new = []
            for inst in bb.instructions:
                si = inst.sync_info
                waits = list(si.on_wait) if si and si.on_wait else []
                if len(waits) > limit:
                    for w in waits[:-limit]:
                        nop = mybir.InstNoOp(name=f"waitsplit-{uid}", ins=[], outs=[])
                        uid += 1
                        nop.engine = inst.engine
                        nop.sync_info = mybir.SyncInfo(on_update=[], on_wait=[w])
                        new.append(nop)
                    si.on_wait.clear()
                    si.on_wait.append(waits[-1])
                new.append(inst)
            bb.instructions = new


def _build_program(nw):
    import bass_rust
    import concourse.bass as bass
    import concourse.mybir as mybir
    from concourse.tile import TileContext

    f32 = mybir.dt.float32
    bf16 = mybir.dt.bfloat16
    AF = mybir.ActivationFunctionType
    V64 = bass_rust.VecI64Pair

    nc = bass.Bass()
    vt = nc.declare_dram_parameter("vt", [E, ROWS], bf16, isOutput=False)
    inpt = nc.declare_dram_parameter("inpt", [E, ROWS], bf16, isOutput=False)
    gw1 = nc.declare_dram_parameter("gw1", [E, H], bf16, isOutput=False)
    gw2 = nc.declare_dram_parameter("gw2", [H, E], bf16, isOutput=False)
    gb1t = nc.declare_dram_parameter("gb1t", [128, HT], f32, isOutput=False)
    gb2r = nc.declare_dram_parameter("gb2r", [1, E], bf16, isOutput=False)
    fw1 = nc.declare_dram_parameter("fw1", [nw, E, H], bf16, isOutput=False)
    fw2t = nc.declare_dram_parameter("fw2t", [nw, 128, HT * NL], bf16, isOutput=False)
    fb1t = nc.declare_dram_parameter("fb1t", [128, nw * HT], f32, isOutput=False)
    fb2c = nc.declare_dram_parameter("fb2c", [NL, nw], f32, isOutput=False)
    onesr = nc.declare_dram_parameter("onesr", [1, E], bf16, isOutput=False)
    out = nc.declare_dram_parameter("out", [N, E2], f32, isOutput=True)

    with TileContext(nc) as tc:
        with (
            tc.tile_pool(name="pc", bufs=1) as pc,
            tc.tile_pool(name="pin", bufs=1) as pin,
            tc.tile_pool(name="pfh", bufs=2) as pfh,
            tc.tile_pool(name="pfw1", bufs=2) as pfw1,
            tc.tile_pool(name="pfw2", bufs=2) as pfw2,
            tc.tile_pool(name="pvtc", bufs=2) as pvtc,
            tc.tile_pool(name="ptmp", bufs=1) as ptmp,
            tc.tile_pool(name="pv", bufs=1) as pv,
            tc.tile_pool(name="ps", bufs=1) as ps,
            tc.tile_pool(name="pwp", bufs=1) as pwp,
            tc.tile_pool(name="pdram", bufs=1, space="DRAM") as pdram,
            tc.tile_pool(name="psA", bufs=2, space="PSUM") as psA,
            tc.tile_pool(name="psW", bufs=2, space="PSUM") as psW,
            tc.tile_pool(name="psO", bufs=1, space="PSUM") as psO,
            tc.tile_pool(name="psC", bufs=3, space="PSUM") as psC,
        ):
            # ---------- DRAM staging tiles (Tile-tracked for cc deps) ----
            va_in = pdram.tile([ROWS, E], bf16, tag="va_in", name="va_in")
            va_out = pdram.tile([2, ROWS, E], bf16, tag="va_out", name="va_out")
            wsis = [
                pdram.tile([NL, ROWS], bf16, tag=f"wsi{m}", name=f"wsi{m}")
                for m in range(nw)
            ]
            wsos = [
                pdram.tile([2, NL, ROWS], bf16, tag=f"wso{m}", name=f"wso{m}")
                for m in range(nw)
            ]
            stage = [
                pdram.tile([6 * 128, PITCH], bf16, tag=f"sst{p}", name=f"sst{p}")
                for p in range(2)
            ]

            # ---------- persistent SBUF tiles ----------
            gw1_t = [pc.tile([128, H], bf16, tag=f"gw1_{k}", name=f"gw1_{k}") for k in range(2)]
            gw2_t = pc.tile([128, HT * E], bf16, tag="gw2", name="gw2")
            gb1_t = pc.tile([128, HT], f32, tag="gb1", name="gb1")
            gb2_t = pc.tile([1, E], bf16, tag="gb2", name="gb2")
            ones_t = pc.tile([1, E], bf16, tag="ones", name="ones")
            fb1_t = pc.tile([128, nw * HT], f32, tag="fb1", name="fb1")
            fb2_t = pc.tile([NL, nw], f32, tag="fb2", name="fb2")
            inp_t = [pin.tile([128, ROWS], bf16, tag=f"inp{k}", name=f"inp{k}") for k in range(2)]

            vcur = pv.tile([128, NBLK * E2], f32, tag="va", name="va")
            vnxt = pv.tile([128, NBLK * E2], f32, tag="vb", name="vb")
            vbf = [pv.tile([128, NBLK * E2], bf16, tag=f"vbf{p}", name=f"vbf{p}") for p in range(2)]
            S = [
                [ps.tile([128, PITCH], bf16, tag=f"s{p}_{k}", name=f"s{p}_{k}") for k in range(6)]
                for p in range(2)
            ]
            Wt = pwp.tile([NL, N], bf16, tag="wt", name="wt")
            wt1 = pwp.tile([NL, N], bf16, tag="wt1", name="wt1")

            # ---------- prologue loads (g-critical first) ----------
            for k in range(2):
                nc.sync.dma_start(out=gw1_t[k][:], in_=gw1[k * 128:(k + 1) * 128, :])
            nc.sync.dma_start(out=gb1_t[:], in_=gb1t[:])
            nc.sync.dma_start(out=gb2_t[:], in_=gb2r[:])
            nc.sync.dma_start(out=ones_t[:], in_=onesr[:])
            for t in range(HT):
                nc.sync.dma_start(
                    out=gw2_t[:, t * E:(t + 1) * E], in_=gw2[t * 128:(t + 1) * 128, :]
                )
            for k in range(2):
                nc.sync.dma_start(out=inp_t[k][:], in_=inpt[k * 128:(k + 1) * 128, :])
            nc.sync.dma_start(out=fb1_t[:], in_=fb1t[:])
            nc.sync.dma_start(out=fb2_t[:], in_=fb2c[:])
            nc.vector.memset(vbf[0][:], 0.0)

            def front_ht(w1k, bias_col, rhs_tiles, fh, ht):
                """One H-tile of an MLP front: 2 matmuls + gelu."""
                pa = psA.tile([128, CH], f32, tag="pa", name="pa")
                for kt in range(2):
                    nc.tensor.matmul(
                        pa[:],
                        lhsT=w1k[kt][:, ht * 128:(ht + 1) * 128],
                        rhs=rhs_tiles[kt],
                        start=(kt == 0),
                        stop=(kt == 1),
                    )
                nc.scalar.activation(fh[ht][:], pa[:], AF.Gelu, bias=bias_col(ht))

            def new_fh():
                return [pfh.tile([128, CH], bf16, tag=f"fh{t}", name=f"fh{t}") for t in range(HT)]

            # ---------- g MLP -> va_in (own rows) ----------
            gfh = {}
            for ch in range(NCH):
                vt_c = [pvtc.tile([128, CH], bf16, tag=f"vtc{k}", name=f"vtc{k}") for k in range(2)]
                for k in range(2):
                    nc.sync.dma_start(
                        out=vt_c[k][:], in_=vt[k * 128:(k + 1) * 128, ch * CH:(ch + 1) * CH]
                    )

                def g_back(cb, t):
                    po = psO.tile([128, E], f32, tag="po", name="po")
                    nc.tensor.matmul(
                        po[:], lhsT=ones_t[0:1, 0:128], rhs=gb2_t[0:1, :],
                        start=True, stop=False,
                    )
                    fhb = gfh[cb]
                    for ht2 in range(HT):
                        nc.tensor.matmul(
                            po[:],
                            lhsT=fhb[ht2][:, t * 128:(t + 1) * 128],
                            rhs=gw2_t[:, ht2 * E:(ht2 + 1) * E],
                            start=False,
                            stop=(ht2 == HT - 1),
                        )
                    tmp = ptmp.tile([128, E], bf16, tag="tv", name="tv")
                    nc.scalar.copy(tmp[:], po[:])
                    blk = cb * 4 + t
                    nc.sync.dma_start(
                        out=va_in[blk * 128:(blk + 1) * 128, :], in_=tmp[:]
                    )

                fh = new_fh()
                gfh[ch] = fh
                for ht in range(HT):
                    front_ht(gw1_t, lambda h2: gb1_t[:, h2:h2 + 1], vt_c, fh, ht)
                    if ch >= 1 and ht % 2 == 1:
                        g_back(ch - 1, ht // 2)
            for t in range(4):
                g_back(NCH - 1, t)

            # pair AllGather of g(V); each core keeps its column half below
            nc.gpsimd.collective_compute(
                "AllGather", mybir.AluOpType.bypass, replica_groups=GROUPS,
                ins=[va_in[:]], outs=[va_out[:]],
            )

            for par in range(2):
                for k in range(6):
                    for hf in range(2):
                        cols = slice(hf * (PITCH // 2), (hf + 1) * (PITCH // 2))
                        nc.sync.dma_start(
                            out=stage[par][k * 128:(k + 1) * 128, cols],
                            in_=vbf[0][:, cols],
                        )

            # ---------- per-layer pieces ----------
            def load_fw(m):
                w1 = [pfw1.tile([128, H], bf16, tag=f"fw1_{k}", name=f"fw1_{k}") for k in range(2)]
                for k in range(2):
                    nc.sync.dma_start(out=w1[k][:], in_=fw1[m, k * 128:(k + 1) * 128, :])
                w2 = pfw2.tile([128, HT * NL], bf16, tag="fw2", name="fw2")
                nc.sync.dma_start(out=w2[:], in_=fw2t[m])
                return w1, w2

            def f_back(m, w2, fh, ch):
                """Back projection of one f chunk: 8 matmuls -> +bias -> DRAM."""
                pw = psW.tile([NL, CH], f32, tag="pw", name="pw")
                for ht in range(HT):
                    nc.tensor.matmul(
                        pw[:],
                        lhsT=w2[:, ht * NL:(ht + 1) * NL],
                        rhs=fh[ht][:],
                        start=(ht == 0),
                        stop=(ht == HT - 1),
                    )
                wc = ptmp.tile([NL, CH], bf16, tag="tw", name="tw")
                nc.vector.tensor_scalar_add(wc[:], pw[:], fb2_t[:, m:m + 1])
                nc.sync.dma_start(out=wsis[m][:, ch * CH:(ch + 1) * CH], in_=wc[:])

            def s_prep_a(m):
                """W load + (j, b)-interleave for layer m (Pool engine)."""
                for h2 in range(2):
                    nc.sync.dma_start(
                        out=wt1[:, h2 * ROWS:(h2 + 1) * ROWS], in_=wsos[m][h2]
                    )
                nc.vector.tensor_copy(
                    Wt[:].rearrange("l (j b) -> l j b", b=NBLK),
                    wt1[:].rearrange("l (b j) -> l j b", j=128),
                )

            def s_prep_b(m, nck=1):
                """Diagonal rewrites of the DRAM staging image + SBUF reload."""
                st = stage[m % 2][:].rearrange("a b -> (a b)")
                Sp = S[m % 2]
                for li, d in enumerate(OFFS):
                    if d <= 128:
                        segs = []
                        if 128 - d > 0:
                            segs.append((0, 0, 128 - d, d))
                        if d > 0:
                            segs.append((1, 128 - d, d, 0))
                    else:
                        si = {256: 2, 512: 3, 1024: 4, 2048: 5}[d]
                        segs = [(si, 0, 128, 0)]
                    for (si, j0, cnt, p0) in segs:
                        src = Wt[li:li + 1, j0 * NBLK:(j0 + cnt) * NBLK]
                        doff = si * 128 * PITCH + p0 * PITCH + j0 * NBLK
                        dst = st[doff:doff + 1]
                        dst.ap = V64([[PITCH + NBLK, cnt], [1, NBLK]])
                        nc.sync.dma_start(out=dst, in_=src)
                for k in range(6):
                    for q4 in range(nck):
                        cols = slice(q4 * (PITCH // nck), (q4 + 1) * (PITCH // nck))
                        nc.sync.dma_start(
                            out=Sp[k][:, cols],
                            in_=stage[m % 2][k * 128:(k + 1) * 128, cols],
                        )

            def chord_quad(k, bq, vc, vn):
                """Output blocks (4bq..4bq+3) of chord layer k: 24 matmuls
                into one bank-wide PSUM tile, f32 residual add (DVE), bf16
                recast (Pool). The last layer streams straight to DRAM."""
                Sp = S[k % 2]
                vb = vbf[k % 2]
                po = psC.tile([128, 4 * E2], f32, tag="pq", name="pq")
                for half in range(4):
                    blk = 4 * bq + half
                    srcs = [(0, blk), (1, (blk + 1) % NBLK)]
                    for i, dl in enumerate([2, 4, 8, 16]):
                        srcs.append((2 + i, (blk + dl) % NBLK))
                    for ii, (si, sb) in enumerate(srcs):
                        nc.tensor.matmul(
                            po[:, half * E2:(half + 1) * E2],
                            lhsT=Sp[si][:, blk::NBLK],
                            rhs=vb[:, sb * E2:(sb + 1) * E2],
                            start=(ii == 0),
                            stop=(ii == 5),
                        )
                sl = slice(bq * 4 * E2, (bq + 1) * 4 * E2)
                nc.vector.tensor_add(vn[:, sl], po[:], vc[:, sl])
                if k + 1 < nw:
                    nc.vector.tensor_copy(vbf[(k + 1) % 2][:, sl], vn[:, sl])
                else:
                    nc.sync.dma_start(
                        out=out[4 * bq * 128:(4 * bq + 4) * 128, :].rearrange(
                            "(b p) e -> p b e", p=128
                        ),
                        in_=vn[:, sl].rearrange("p (b e) -> p b e", e=E2),
                    )

            def cc_w(m):
                nc.gpsimd.collective_compute(
                    "AllGather", mybir.AluOpType.bypass, replica_groups=GROUPS,
                    ins=[wsis[m][:]], outs=[wsos[m][:]],
                )

            def f_step(m, w1, w2, quads):
                """One pipeline step: f-MLP_m at chunk granularity with chord
                quads as PE spacers; the W AllGather fires before the
                trailing quads so its latency hides upstream."""
                qi = 0
                for ch in range(NCH):
                    fh = new_fh()
                    rhs = [inp_t[k][:, ch * CH:(ch + 1) * CH] for k in range(2)]
                    for ht in range(HT):
                        front_ht(
                            w1, lambda h2: fb1_t[:, m * HT + h2:m * HT + h2 + 1],
                            rhs, fh, ht,
                        )
                    if ch < NCH - 1 and qi < len(quads):
                        quads[qi]()
                        qi += 1
                    f_back(m, w2, fh, ch)
                    if ch < NCH - 1 and qi < len(quads):
                        quads[qi]()
                        qi += 1
                cc_w(m)
                for q in quads[qi:]:
                    q()

            # ---------- fused pipeline ----------
            # step m: f_m || chord_{m-3} || s_prep(m-2); lag 3 keeps every
            # wait (cc completion, S reload) at least one full step upstream.
            vc, vn = vcur, vnxt
            w1n, w2n = load_fw(0)
            for m in range(nw):
                w1, w2 = w1n, w2n
                if m + 1 < nw:
                    w1n, w2n = load_fw(m + 1)
                if m >= 2:
                    s_prep_a(m - 2)
                quads = []
                if m >= 3:
                    quads = [
                        (lambda k=m - 3, b=bq, c=vc, n=vn: chord_quad(k, b, c, n))
                        for bq in range(8)
                    ]
                f_step(m, w1, w2, quads)
                if m >= 2:
                    s_prep_b(m - 2, nck=(4 if m - 2 == nw - 3 else 1))
                if m >= 3:
                    vc, vn = vn, vc
                if m == 0:
                    # initial V load: pick my column half of the gathered
                    # g(V) with conditional DMAs keyed on core parity
                    eh = nc.sync.partition_id() % 2
                    dv = vbf[0][:].rearrange("p (blk e) -> p blk e", e=E2)
                    for par in range(2):
                        sv = va_out[:].rearrange("a b c -> (a b c)")[par * E2:par * E2 + 1]
                        sv.ap = V64([[E, 128], [128 * E, NBLK], [1, E2]])
                        nc.sync.dma_start(out=dv, in_=sv, cond=(eh == par))
                    for q in range(4):
                        nc.vector.tensor_copy(
                            vcur[:, q * 8 * E2:(q + 1) * 8 * E2],
                            vbf[0][:, q * 8 * E2:(q + 1) * 8 * E2],
                        )

            # tail: three chord-only steps with next-layer S prep interleaved
            for k in range(nw - 3, nw):
                if k + 1 < nw:
                    s_prep_a(k + 1)
                for bq in range(8):
                    chord_quad(k, bq, vc, vn)
                    if bq == 1 and k + 1 < nw:
                        s_prep_b(k + 1, nck=4)
                vc, vn = vn, vc

    _split_multi_waits(nc, mybir)
    return nc


def kernel(**inputs):
    _install_patches()
    from concourse.bass_utils import run_bass_kernel_spmd

    nw = int(os.environ.get("K_NW", NW))
    V = np.ascontiguousarray(np.asarray(inputs["V"], dtype=np.float32))
    inp = np.ascontiguousarray(np.asarray(inputs["input"], dtype=np.float32))
    g_W1 = np.ascontiguousarray(np.asarray(inputs["g_W1"], dtype=np.float32))
    g_b1 = np.asarray(inputs["g_b1"], dtype=np.float32)
    g_W2 = np.ascontiguousarray(np.asarray(inputs["g_W2"], dtype=np.float32))
    g_b2 = np.asarray(inputs["g_b2"], dtype=np.float32)
    f_W1 = np.ascontiguousarray(np.asarray(inputs["f_W1"], dtype=np.float32))[:nw]
    f_b1 = np.asarray(inputs["f_b1"], dtype=np.float32)[:nw]
    f_W2 = np.ascontiguousarray(np.asarray(inputs["f_W2"], dtype=np.float32))[:nw]
    f_b2 = np.asarray(inputs["f_b2"], dtype=np.float32)[:nw]

    import ml_dtypes

    bf = ml_dtypes.bfloat16
    gb1t = np.ascontiguousarray(g_b1.reshape(HT, 128).T)
    fw2t = np.ascontiguousarray(
        f_W2.reshape(nw, HT, 128, NL).transpose(0, 2, 1, 3).reshape(nw, 128, HT * NL)
    ).astype(bf)
    fb1t = np.ascontiguousarray(
        f_b1.reshape(nw, HT, 128).transpose(2, 0, 1).reshape(128, nw * HT)
    )
    fb2c = np.ascontiguousarray(f_b2.T)

    shared = {
        "gw1": g_W1.astype(bf),
        "gw2": g_W2.astype(bf),
        "gb1t": gb1t,
        "gb2r": np.ascontiguousarray(g_b2[None, :]).astype(bf),
        "onesr": np.ones((1, E), bf),
        "fw1": f_W1.astype(bf),
        "fw2t": fw2t,
        "fb1t": fb1t,
        "fb2c": fb2c,
    }
    in_maps = []
    for c in range(8):
        b, h = c // 2, c % 2
        rows = slice(h * ROWS, (h + 1) * ROWS)
        m = dict(shared)
        m["vt"] = np.ascontiguousarray(V[b, rows].T).astype(bf)
        m["inpt"] = np.ascontiguousarray(inp[b, rows].T).astype(bf)
        in_maps.append(m)

    nc = _build_program(nw)
    trace = bool(int(os.environ.get("K_TRACE", "0")))
    res = run_bass_kernel_spmd(nc, in_maps, list(range(8)), trace=trace)
    kernel.last_result = res

    outp = np.empty((B, N, E), np.float32)
    for b in range(B):
        outp[b, :, 0:E2] = res.results[2 * b]["out"]
        outp[b, :, E2:E] = res.results[2 * b + 1]["out"]
    return outp
